# revision 1
# baseline (speedup 1.0000x reference)
"""2D DCT-II (ortho) on (32, 3, 512, 512) fp32, data-parallel across 8 TRN2 NeuronCores.

Quadrant-folded formulation. Using D[k, 511-n] = (-1)^k D[k, n] on BOTH axes:
  out[2a+ph, 2b+pw] = sum_{n'<256, w'<256} Dh[a,n'] Dw[b,w'] XQ[n',w']
where XQ is the (ph, pw) parity fold of X and Dh/Dw are the even/odd rows of
D restricted to the first 256 columns. This halves the matmul work vs a
W-only fold and keeps every PE matmul at 256 free columns.

All device data is bf16 (fp32 PSUM accumulation): halves HBM traffic vs
fp32; absmax-scaled error ~5e-3 against a 2e-2 budget. The host shards,
computes the four O(N^2) parity folds during input staging (the device
cannot fold across partitions: DMA rejects negative partition strides and
GPSIMD measured 2.07x slower than DVE, starving the PE), and converts the
bf16 output back to fp32.

Per image on device (steady state, ~3.8 us):
  sync  4 input DMAs (one per quadrant, 128x1KB lines) + 2 store dispatches
  PE    pass A 16 MM + pass B 16 MM, all free=256 bf16, ~110 ns/slot
        (stream-limited; stride-2 PSUM matmul writes measured 2x slower, so
        pass B writes contiguous halves and the copy-out interleaves)
  ACT   4 psumA->SBUF bf16 copies (the pass A -> pass B transpose staging)
  DVE   4 psumB->SBUF bf16 interleaving copies (full output rows)
  GP    4 store dispatches (one per row-parity block)
"""
import os
import sys

for _p in ("/opt/trn_rl_repo", os.path.expanduser("~/.axon_site/_ro/trn_rl_repo")):
    if os.path.isdir(_p) and _p not in sys.path:
        sys.path.insert(0, _p)

import numpy as np
import ml_dtypes
import concourse.bass as bass
import concourse.bacc as bacc
import concourse.mybir as mybir
import concourse.tile as tile
from concourse.bass_utils import run_bass_kernel_spmd

dt = mybir.dt
BF = ml_dtypes.bfloat16

N = 512            # image height/width
H = N // 2         # 256 folded size
P = 128            # SBUF partitions
N_CORES = 8
B, CH = 32, 3
IMGS = (B * CH) // N_CORES  # 12 images per core


def _consts() -> tuple[np.ndarray, np.ndarray]:
    n = np.arange(N, dtype=np.float64)
    k = n[:, None]
    D = np.cos(np.pi * (2.0 * n[None, :] + 1.0) * k / (2.0 * N))
    D[0] *= np.sqrt(1.0 / N)
    D[1:] *= np.sqrt(2.0 / N)
    det = np.ascontiguousarray(D[0::2, :H].T).astype(BF)  # [n', a]
    dot = np.ascontiguousarray(D[1::2, :H].T).astype(BF)
    return det, dot


def _build_nc() -> bacc.Bacc:
    nc = bacc.Bacc("TRN2", target_bir_lowering=False, debug=False, num_devices=N_CORES)
    # xq[i, q, p, c, w']: quadrant q=2*ph+pw of image i, row n'=128c+p
    xq = nc.dram_tensor("xq", [IMGS, 4, P, 2, H], dt.bfloat16, kind="ExternalInput")
    out = nc.dram_tensor("out", [IMGS, N, N], dt.bfloat16, kind="ExternalOutput")
    det_t = nc.dram_tensor("det_t", [H, H], dt.bfloat16, kind="ExternalInput")
    dot_t = nc.dram_tensor("dot_t", [H, H], dt.bfloat16, kind="ExternalInput")

    bf16 = dt.bfloat16
    f32 = dt.float32

    with tile.TileContext(nc) as tc:
        with (
            tc.tile_pool(name="const", bufs=1) as const_pool,
            tc.tile_pool(name="qd", bufs=3) as q_pool,
            tc.tile_pool(name="pq", bufs=2) as pq_pool,
            tc.tile_pool(name="res", bufs=2) as res_pool,
            tc.tile_pool(name="psa", bufs=3, space="PSUM") as psa_pool,
            tc.tile_pool(name="psb", bufs=5, space="PSUM") as psb_pool,
        ):
            # DhT[n', a] tiles: dh[p, 256c + a] = Dh[a, 128c + p]
            det_sb = const_pool.tile([P, 2 * H], bf16)
            nc.scalar.dma_start(
                det_sb[:].rearrange("p (c a) -> p c a", c=2),
                det_t.ap().rearrange("(c p) a -> p c a", p=P),
            )
            dot_sb = const_pool.tile([P, 2 * H], bf16)
            nc.scalar.dma_start(
                dot_sb[:].rearrange("p (c a) -> p c a", c=2),
                dot_t.ap().rearrange("(c p) a -> p c a", p=P),
            )
            dh_sb = {0: det_sb, 1: dot_sb}

            # PE warmup during the DMA ramp (pstate clock gate)
            scr = const_pool.tile([P, N + P], bf16)
            nc.gpsimd.memset(scr[:], 0.0)
            ps_w = psb_pool.tile([P, N], f32, tag="psB")
            for _ in range(8):
                nc.tensor.matmul(
                    ps_w[:], scr[:, N : N + P], scr[:, :N], start=True, stop=True
                )

            for i in range(IMGS):
                # per-quadrant loads: qt[q][p, 256c + w'] = XQ[128c+p, w']
                # image 0's four loads spread across engines so they dispatch
                # in parallel right after the startup barrier
                load_eng = None  # all loads on sync: scalar/gpsimd queues are busy at the ramp
                qt = {}
                for ph in (0, 1):
                    for pw in (0, 1):
                        q = q_pool.tile(
                            [P, 2 * H], bf16, tag=f"q{ph}{pw}", name=f"q{ph}{pw}_{i}"
                        )
                        eng = load_eng[2 * ph + pw] if load_eng else nc.sync
                        eng.dma_start(
                            q[:].rearrange("p (c j) -> p c j", c=2),
                            xq.ap()[i][2 * ph + pw],
                        )
                        qt[(ph, pw)] = q

                # pass A: P_Q[w', a] = sum_{n'} XQ[n', w'] Dh[a, n']
                # psA[p, 256wb + a] = P_Q[128wb + p, a]
                pq = {}
                for ph in (0, 1):
                    for pw in (0, 1):
                        psA = psa_pool.tile([P, N], f32, tag="psA")
                        q = qt[(ph, pw)]
                        dh = dh_sb[ph]
                        for wb in (0, 1):
                            for c in (0, 1):
                                nc.tensor.matmul(
                                    psA[:, H * wb : H * (wb + 1)],
                                    q[:, H * c + P * wb : H * c + P * wb + P],
                                    dh[:, H * c : H * (c + 1)],
                                    start=(c == 0), stop=(c == 1),
                                )
                        t = pq_pool.tile(
                            [P, N], bf16, tag=f"pq{ph}{pw}", name=f"pq{ph}{pw}_{i}"
                        )
                        nc.scalar.copy(t[:], psA[:])
                        pq[(ph, pw)] = t

                # pass B: out[2(128ab+p)+ph, 2b+pw] = sum_{w'} P_Q[w',a] Dw[b,w']
                # psB columns pre-interleaved: psB[p, 2b + pw]
                o_t = [
                    res_pool.tile([P, 2 * N], bf16, tag=f"o{ab}", name=f"o{ab}_{i}")
                    for ab in (0, 1)
                ]
                for ab in (0, 1):
                    for ph in (0, 1):
                        psB = psb_pool.tile([P, N], f32, tag="psB")
                        for pw in (0, 1):
                            for wc in (0, 1):
                                nc.tensor.matmul(
                                    psB[:, H * pw : H * (pw + 1)],
                                    pq[(ph, pw)][:, H * wc + P * ab : H * wc + P * ab + P],
                                    dh_sb[pw][:, H * wc : H * (wc + 1)],
                                    start=(wc == 0), stop=(wc == 1),
                                )
                        # interleave during the copy: o[p, 512ph + 2b + pw] = psB[p, 256pw + b]
                        oap = o_t[ab][:]
                        dst = bass.AP(
                            oap.tensor, oap.offset + N * ph,
                            [[oap.ap[0][0], P], [1, 2], [2, H]],
                        )
                        nc.vector.tensor_copy(
                            dst, psB[:].rearrange("p (w b) -> p w b", w=2)
                        )
                        # store rows 256ab + 2p + ph right after the copy.
                        # gpsimd is idle so it dispatches, EXCEPT the last
                        # image: its stores go to sync (loads done) so the
                        # gpsimd DMA queue is empty before its ~5us end-drain
                        eng = nc.sync if i == IMGS - 1 else nc.gpsimd
                        eng.dma_start(
                            bass.AP(
                                out, i * N * N + (2 * P * ab + ph) * N,
                                [[2 * N, P], [1, N]],
                            ),
                            o_t[ab][:, N * ph : N * (ph + 1)],
                        )

    nc.compile()
    return nc


_NC_CACHE: bacc.Bacc | None = None


def _get_nc() -> bacc.Bacc:
    global _NC_CACHE
    if _NC_CACHE is None:
        _NC_CACHE = _build_nc()
    return _NC_CACHE


def _fold_quadrants(xs: np.ndarray) -> np.ndarray:
    """[IMGS, 512, 512] fp32 -> [IMGS, 4, 128, 2, 256] bf16 parity quadrants."""
    lo, hi = xs[:, :H], xs[:, N - 1 : H - 1 : -1]  # hi rows reversed
    he, ho = lo + hi, lo - hi                      # [IMGS, 256, 512]
    quads = np.empty((IMGS, 4, H, H), np.float32)
    for ph, s in ((0, he), (1, ho)):
        wl, wr = s[:, :, :H], s[:, :, N - 1 : H - 1 : -1]
        quads[:, 2 * ph + 0] = wl + wr
        quads[:, 2 * ph + 1] = wl - wr
    # row n' = 128c + p -> [i, q, p, c, w']
    return np.ascontiguousarray(
        quads.reshape(IMGS, 4, 2, P, H).transpose(0, 1, 3, 2, 4)
    ).astype(BF)


def run(inp: np.ndarray, **spmd_kwargs):
    """Shard, fold, run on 8 cores, gather. Returns (output, BassKernelResults)."""
    x = np.asarray(inp, dtype=np.float32)
    assert x.shape == (B, CH, N, N), x.shape
    shards = x.reshape(N_CORES, IMGS, N, N)
    det, dot = _consts()
    in_maps = [
        {"xq": _fold_quadrants(shards[c]), "det_t": det, "dot_t": dot}
        for c in range(N_CORES)
    ]
    res = run_bass_kernel_spmd(_get_nc(), in_maps, core_ids=list(range(N_CORES)), **spmd_kwargs)
    out = np.stack([res.results[c]["out"] for c in range(N_CORES)])
    return out.reshape(B, CH, N, N).astype(np.float32), res


def kernel(inp: np.ndarray) -> np.ndarray:
    out, _ = run(inp)
    return out

